# revision 61
# baseline (speedup 1.0000x reference)
"""Distributed AttentionBlock kernel for 8 TRN2 NeuronCores.

Sharding: tensor-parallel over heads (16 heads -> 2 per core) for
qkv-projection + attention; per-batch AllToAll redistributes attention
output so each core computes the out-projection for a 512-token slice of
EACH batch; host-side unshard is a pure concat.

Host side pre-transposes x (-> [C, T]) and the weight matrices, so the
device never transposes activations except v (xbar DMA transpose).

Per-core pipeline (all matmuls bf16 inputs, fp32 accumulate):
  xT --DMA--> sbuf f32 -> DVE cast bf16
  qkvT[dim,tok] = wT.T @ xT  (wT pre-transposed on host)
  v re-transposed token-major via xbar DMA into V_aug
  S^T[m,q] = kT.T @ qT       (two row-tiled K=64 matmuls)
  P = exp(S^T/8)             (ScalarE from PSUM; no max subtraction:
                              |scores| <= ~3 for this distribution)
  O_aug[65,q] = V_aug.T @ P  (V ones-cols -> row 64 = softmax denominator)
  AllToAll#b (per batch) ships unnormalized O + denominators (gpsimd
  queue so the sync queue never stalls on attention results);
  recv side: K=1 ones-matmul broadcast + reciprocal -> normalize ->
  out-projection (+ bias via K=1 ones matmul).

Program order: batch 1's qkv groups are interleaved into batch 0's
attention (engine queues are FIFO); batch 0's tail is emitted mid-way
through batch 1's attention so its A2A + out-projection hide under
attention compute.

Constraint: hidden == 128 * n_cores (head_dim 64, 2 heads per core).
Full size: n_cores=8, hidden=1024, tokens/batch=4096.
"""

import numpy as np

HIDDEN = 1024
HEAD_DIM = 64
N_CORES = 8
B = 2


def build_nc(n_tok_b=4096, n_cores=8, hidden=1024):
    import concourse.bacc as bacc
    import concourse.tile as tile
    import concourse.mybir as mybir

    f32 = mybir.dt.float32
    bf16 = mybir.dt.bfloat16
    AF = mybir.ActivationFunctionType
    ALU = mybir.AluOpType

    C = hidden
    CS = C // 128            # contraction slices == n_cores
    assert CS == n_cores
    NB = n_tok_b
    GRP = 512                # qkv token-group
    NGRP = NB // GRP
    NMB = NB // 128          # key blocks per batch
    QC = 512                 # query chunk == slice tokens per core per batch
    NQC = NB // QC
    assert NQC == n_cores
    OTB = QC // 128          # out-proj token blocks per batch

    nc = bacc.Bacc("TRN2", target_bir_lowering=False, debug=False,
                   num_devices=n_cores)

    xt_d = nc.declare_dram_parameter("xt", [C, B * NB], f32, isOutput=False)
    qkvwt_d = nc.declare_dram_parameter("qkvwt", [3, C, 128], f32,
                                        isOutput=False)
    qkvb_d = nc.declare_dram_parameter("qkvb", [3, 128, 1], f32, isOutput=False)
    outwt_d = nc.declare_dram_parameter("outwt", [C, C], f32, isOutput=False)
    outb_d = nc.declare_dram_parameter("outb", [1, C], f32, isOutput=False)
    out_d = nc.declare_dram_parameter("out", [B * QC, C], f32, isOutput=True)

    # A2A payload rows 0:128 = unnormalized O (2 heads); 128:130 = denoms
    binc = [nc.dram_tensor(f"binc{b}", [n_cores, 130, QC], bf16)
            for b in range(B)]
    bout = [nc.dram_tensor(f"bout{b}", [n_cores, 130, QC], bf16)
            for b in range(B)]

    with tile.TileContext(nc) as tc:
        with (
            tc.tile_pool(name="persist", bufs=1) as pp,
            tc.tile_pool(name="xload", bufs=2) as xp,
            tc.tile_pool(name="xt", bufs=2) as xtp,
            tc.tile_pool(name="pexp", bufs=3) as pexpp,
            tc.tile_pool(name="misc", bufs=2) as mp,
            tc.tile_pool(name="recvp", bufs=1) as rp,
            tc.tile_pool(name="scratch", bufs=2, space="PSUM") as scr,
            tc.tile_pool(name="stp", bufs=2, space="PSUM") as stp,
            tc.tile_pool(name="op", bufs=2, space="PSUM") as op,
        ):
            # ---- weights: load host-pre-transposed, cast to bf16 ----
            wT = [pp.tile([128, CS, 128], bf16, tag=f"wT{m}", name=f"wT{m}")
                  for m in range(3)]
            for m in range(3):
                wld = xp.tile([128, CS, 128], f32, tag="wld")
                nc.sync.dma_start(
                    wld[:],
                    qkvwt_d[m].rearrange("(cs p) d -> p cs d", p=128))
                nc.vector.tensor_copy(wT[m][:], wld[:])

            owT = pp.tile([128, CS, C], bf16, tag="owT")
            for g in range(CS):
                owld = xp.tile([128, C], f32, tag="owld")
                nc.sync.dma_start(owld[:], outwt_d[g * 128:(g + 1) * 128, :])
                nc.vector.tensor_copy(owT[:, g, :], owld[:])

            bias_sb = pp.tile([128, 3], f32, tag="bias")
            for m in range(3):
                nc.sync.dma_start(bias_sb[:, m:m + 1], qkvb_d[m])
            outb_f = pp.tile([1, C], f32, tag="outbf")
            nc.sync.dma_start(outb_f[:], outb_d[:])
            outb_sb = pp.tile([1, C], bf16, tag="outb")
            nc.vector.tensor_copy(outb_sb[:], outb_f[:])
            ones_sb = pp.tile([1, 128], bf16, tag="ones")
            nc.vector.memset(ones_sb[:], 1.0)

            # ---- per-batch persistent tensors ----
            qT = [pp.tile([128, NB], bf16, tag=f"qT{b}", name=f"qT{b}")
                  for b in range(B)]
            kT = [pp.tile([128, NB], bf16, tag=f"kT{b}", name=f"kT{b}")
                  for b in range(B)]
            # V layout per key-block: [h0 d0..63, ones, h1 d0..63, ones]
            # -> O-matmul output row 64 is the softmax denominator
            V = [pp.tile([128, NMB, 130], bf16, tag=f"V{b}", name=f"V{b}")
                 for b in range(B)]
            for b in range(B):
                nc.vector.memset(V[b][:], 1.0)

            def qkv_group(b, grp):
                tok0 = b * NB + grp * GRP
                xlf = xp.tile([128, CS, GRP], f32, tag="xlf")
                nc.sync.dma_start(
                    xlf[:],
                    xt_d[:, tok0:tok0 + GRP].rearrange(
                        "(cs p) t -> p cs t", p=128))
                xt = xtp.tile([128, CS, GRP], bf16, tag="xt")
                nc.vector.tensor_copy(xt[:], xlf[:])
                for m in range(3):
                    qp = scr.tile([128, GRP], f32, tag="s")
                    for cs in range(CS):
                        nc.tensor.matmul(
                            qp[:], wT[m][:, cs, :], xt[:, cs, :],
                            start=(cs == 0), stop=(cs == CS - 1))
                    if m < 2:
                        dest = (qT if m == 0 else kT)[b][
                            :, grp * GRP:(grp + 1) * GRP]
                        nc.vector.tensor_scalar(dest, qp[:],
                                                bias_sb[:, m:m + 1],
                                                None, op0=ALU.add)
                    else:
                        vs = mp.tile([128, GRP], bf16, tag="vs")
                        nc.vector.tensor_scalar(vs[:], qp[:],
                                                bias_sb[:, 2:3],
                                                None, op0=ALU.add)
                        mb0 = grp * (GRP // 128)
                        # xbar transpose (full-128-partition source), then
                        # two strided DVE copies split the heads into V
                        vt = mp.tile([128, GRP // 128, 128], bf16, tag="vt")
                        nc.sync.dma_start_transpose(vt[:], vs[:])
                        nc.vector.tensor_copy(
                            V[b][:, mb0:mb0 + 4, 0:64], vt[:, :, 0:64])
                        nc.vector.tensor_copy(
                            V[b][:, mb0:mb0 + 4, 65:129], vt[:, :, 64:128])

            def attn_phase(b, mid_hook=None, per_qc_hook=None):
                for qc in range(NQC):
                    if per_qc_hook is not None:
                        per_qc_hook(qc)
                    oh0 = op.tile([65, QC], f32, tag="oh")
                    oh1 = op.tile([65, QC], f32, tag="oh")
                    # O-matmuls lag the S-matmuls by 2 iterations so the PE
                    # FIFO never stalls waiting for the exp of the current
                    # score tile (ACT ~1.1us > S-pair ~0.6us)
                    pes = {}
                    for mb in range(NMB + 2):
                        if mb < NMB:
                            st = stp.tile([128, 2 * QC], f32, tag="st")
                            for h in range(2):
                                nc.tensor.matmul(
                                    st[:, h * QC:(h + 1) * QC],
                                    kT[b][64 * h:64 * h + 64,
                                          mb * 128:mb * 128 + 128],
                                    qT[b][64 * h:64 * h + 64,
                                          qc * QC:(qc + 1) * QC],
                                    start=True, stop=True)
                            pe = pexpp.tile([128, 2 * QC], bf16, tag="pe")
                            nc.scalar.activation(pe[:], st[:], AF.Exp,
                                                 scale=0.125)
                            pes[mb] = pe
                        if mb >= 2:
                            mo = mb - 2
                            pe = pes.pop(mo)
                            nc.tensor.matmul(oh0[:], V[b][:, mo, 0:65],
                                             pe[:, 0:QC],
                                             start=(mo == 0),
                                             stop=(mo == NMB - 1))
                            nc.tensor.matmul(oh1[:], V[b][:, mo, 65:130],
                                             pe[:, QC:2 * QC],
                                             start=(mo == 0),
                                             stop=(mo == NMB - 1))
                    for h, oh in ((0, oh0), (1, oh1)):
                        # evacuation DMAs ride the idle gpsimd queue so the
                        # sync queue (x loads) never stalls on attention
                        ohs = mp.tile([64, QC], bf16, tag="ohs")
                        nc.vector.tensor_copy(ohs[:], oh[0:64, :])
                        nc.gpsimd.dma_start(
                            binc[b][qc, h * 64:(h + 1) * 64, :], ohs[:])
                        ds = mp.tile([1, QC], bf16, tag="ds")
                        nc.vector.tensor_copy(ds[:], oh[64:65, :])
                        nc.gpsimd.dma_start(
                            binc[b][qc, 128 + h:129 + h, :], ds[:])
                    if mid_hook is not None and qc == NQC // 2 - 1:
                        mid_hook()

            def a2a_phase(b):
                nc.gpsimd.collective_compute(
                    "AllToAll", ALU.bypass,
                    replica_groups=[list(range(n_cores))],
                    ins=[binc[b].ap().opt()],
                    outs=[bout[b].ap().opt()],
                )

            tail_state = {}

            def tail_norm(b, use_scalar_recip=False):
                recv = rp.tile([128, n_cores * QC], bf16, tag="recv")
                for g in range(n_cores):
                    nc.sync.dma_start(recv[:, g * QC:(g + 1) * QC],
                                      bout[b][g, 0:128, :])
                recvd = rp.tile([1, n_cores * 2 * QC], bf16, tag="recvd")
                nc.sync.dma_start(
                    recvd[:].rearrange("p (g h q) -> p g h q", g=n_cores, h=2),
                    bout[b][:, 128:130, :])
                rnorm = rp.tile([128, n_cores * QC], bf16, tag="rnorm")
                for g in range(n_cores):
                    # K=1 ones-matmul broadcasts the two denominator rows
                    # across partitions 0:64 / 64:128 of a PSUM tile
                    rb = scr.tile([128, QC], f32, tag="s")
                    for h in range(2):
                        nc.tensor.matmul(
                            rb[h * 64:(h + 1) * 64, :],
                            ones_sb[0:1, 0:64],
                            recvd[0:1,
                                  (g * 2 + h) * QC:(g * 2 + h + 1) * QC],
                            start=True, stop=True)
                    rcp = mp.tile([128, QC], f32, tag="rcp")
                    if use_scalar_recip:
                        # ScalarE is idle after the last exp: 1/x via
                        # exp(-ln(x)) is ~2x faster than the DVE's
                        # iterative divide and stays off the DVE chain
                        rln = mp.tile([128, QC], f32, tag="rln")
                        nc.scalar.activation(rln[:], rb[:], AF.Ln)
                        nc.scalar.activation(rcp[:], rln[:], AF.Exp,
                                             scale=-1.0)
                    else:
                        nc.vector.reciprocal(rcp[:], rb[:])
                    nc.vector.scalar_tensor_tensor(
                        rnorm[:, g * QC:(g + 1) * QC],
                        recv[:, g * QC:(g + 1) * QC], 1.0, rcp[:],
                        op0=ALU.mult, op1=ALU.mult)
                tail_state[b] = rnorm

            def tail_out(b, tb_lo, tb_hi):
                rnorm = tail_state[b]
                for tb in range(tb_lo, tb_hi):
                    ot = mp.tile([128, C], f32, tag="ot")
                    for co2 in range(C // 512):
                        pj = scr.tile([128, 512], f32, tag="s")
                        for g in range(n_cores):
                            nc.tensor.matmul(
                                pj[:],
                                rnorm[:, g * QC + tb * 128:
                                      g * QC + tb * 128 + 128],
                                owT[:, g, co2 * 512:(co2 + 1) * 512],
                                start=(g == 0), stop=False)
                        nc.tensor.matmul(pj[:], ones_sb[:],
                                         outb_sb[:, co2 * 512:(co2 + 1) * 512],
                                         start=False, stop=True)
                        nc.vector.tensor_copy(ot[:, co2 * 512:(co2 + 1) * 512],
                                              pj[:])
                    nc.sync.dma_start(
                        out_d[b * QC + tb * 128:b * QC + (tb + 1) * 128, :],
                        ot[:])

            def attn1_hook(qc):
                # spread batch 0's tail over several qc slots so its PE
                # work doesn't pile onto single attention iterations
                if qc == 3:
                    tail_norm(0)
                elif qc == 4:
                    tail_out(0, 0, 2)
                elif qc == 5:
                    tail_out(0, 2, OTB)

            for grp in range(NGRP):
                qkv_group(0, grp)
            for grp in range(NGRP):
                qkv_group(1, grp)
            attn_phase(0)
            a2a_phase(0)
            attn_phase(1, per_qc_hook=attn1_hook)
            a2a_phase(1)
            tail_norm(1)
            tail_out(1, 0, OTB)

    nc.compile()
    return nc


def shard_inputs(x, qkv_w, qkv_b, out_w, out_b, n_cores=8):
    """Per-core input maps. hidden == 128*n_cores; core c owns qkv rows
    [c*128, (c+1)*128) of each of q, k, v. x and the weights are
    pre-transposed on the host so the device needs no transposes."""
    Bv, N, Cc = x.shape
    T = Bv * N
    xth = np.ascontiguousarray(x.reshape(T, Cc).T, dtype=np.float32)
    owt = np.ascontiguousarray(out_w.T, dtype=np.float32)
    ob = np.ascontiguousarray(out_b.reshape(1, Cc), dtype=np.float32)
    in_maps = []
    for c in range(n_cores):
        r0 = c * 128
        wt = np.stack([
            np.ascontiguousarray(qkv_w[m * Cc + r0: m * Cc + r0 + 128].T)
            for m in range(3)])
        bvec = np.stack([qkv_b[m * Cc + r0: m * Cc + r0 + 128]
                         for m in range(3)])[:, :, None]
        in_maps.append({
            "xt": xth,
            "qkvwt": np.ascontiguousarray(wt.astype(np.float32)),
            "qkvb": np.ascontiguousarray(bvec.astype(np.float32)),
            "outwt": owt, "outb": ob,
        })
    return in_maps


def unshard_output(results, n_cores=8, n_tok_b=4096, hidden=1024):
    """results[c]["out"] is [B*512, C]: rows 0:512 = batch-0 slice c,
    rows 512:1024 = batch-1 slice c."""
    QC = 512
    out = np.empty((B, n_tok_b, hidden), dtype=np.float32)
    for c in range(n_cores):
        r = results[c]["out"]
        for b in range(B):
            out[b, c * QC:(c + 1) * QC] = r[b * QC:(b + 1) * QC]
    return out


_NC_CACHE = {}


def kernel(x, qkv_w, qkv_b, out_w, out_b):
    from concourse import bass_utils
    x = np.asarray(x)
    Bv, N, Cc = x.shape
    key = (N, Cc)
    if key not in _NC_CACHE:
        _NC_CACHE[key] = build_nc(n_tok_b=N, n_cores=N_CORES, hidden=Cc)
    nc = _NC_CACHE[key]
    in_maps = shard_inputs(x, np.asarray(qkv_w), np.asarray(qkv_b),
                           np.asarray(out_w), np.asarray(out_b),
                           n_cores=N_CORES)
    res = bass_utils.run_bass_kernel_spmd(nc, in_maps,
                                          core_ids=list(range(N_CORES)))
    out = unshard_output([res.results[i] for i in range(N_CORES)],
                         n_cores=N_CORES, n_tok_b=N, hidden=Cc)
    return out.astype(np.float32)


# revision 65
# speedup vs baseline: 1.0689x; 1.0689x over previous
"""Distributed AttentionBlock kernel for 8 TRN2 NeuronCores.

Sharding: tensor-parallel over heads (16 heads -> 2 per core) for
qkv-projection + attention; per-batch AllToAll redistributes attention
output so each core computes the out-projection for a 512-token slice of
EACH batch; host-side unshard is a pure concat.

Host side pre-transposes x (-> [C, T]) and the weight matrices, so the
device never transposes activations except v (xbar DMA transpose).

Per-core pipeline (all matmuls bf16 inputs, fp32 accumulate):
  xT --DMA--> sbuf f32 -> DVE cast bf16
  qkvT[dim,tok] = wT.T @ xT  (wT pre-transposed on host)
  v re-transposed token-major via xbar DMA into V_aug
  S^T[m,q] = kT.T @ qT       (two row-tiled K=64 matmuls)
  P = exp(S^T/8)             (ScalarE from PSUM; no max subtraction:
                              |scores| <= ~3 for this distribution)
  O_aug[65,q] = V_aug.T @ P  (V ones-cols -> row 64 = softmax denominator)
  AllToAll#b (per batch) ships unnormalized O + denominators (gpsimd
  queue so the sync queue never stalls on attention results);
  recv side: K=1 ones-matmul broadcast + reciprocal -> normalize ->
  out-projection (+ bias via K=1 ones matmul).

Program order: batch 1's qkv groups are interleaved into batch 0's
attention (engine queues are FIFO); batch 0's tail is emitted mid-way
through batch 1's attention so its A2A + out-projection hide under
attention compute.

Constraint: hidden == 128 * n_cores (head_dim 64, 2 heads per core).
Full size: n_cores=8, hidden=1024, tokens/batch=4096.
"""

import numpy as np

HIDDEN = 1024
HEAD_DIM = 64
N_CORES = 8
B = 2


def build_nc(n_tok_b=4096, n_cores=8, hidden=1024):
    import concourse.bacc as bacc
    import concourse.tile as tile
    import concourse.mybir as mybir

    f32 = mybir.dt.float32
    bf16 = mybir.dt.bfloat16
    AF = mybir.ActivationFunctionType
    ALU = mybir.AluOpType

    C = hidden
    CS = C // 128            # contraction slices == n_cores
    assert CS == n_cores
    NB = n_tok_b
    GRP = 512                # qkv token-group
    NGRP = NB // GRP
    NMB = NB // 128          # key blocks per batch
    QC = 512                 # query chunk == slice tokens per core per batch
    NQC = NB // QC
    assert NQC == n_cores
    OTB = QC // 128          # out-proj token blocks per batch

    nc = bacc.Bacc("TRN2", target_bir_lowering=False, debug=False,
                   num_devices=n_cores)

    xt_d = nc.declare_dram_parameter("xt", [C, B * NB], f32, isOutput=False)
    qkvwt_d = nc.declare_dram_parameter("qkvwt", [3, C, 128], f32,
                                        isOutput=False)
    qkvb_d = nc.declare_dram_parameter("qkvb", [3, 128, 1], f32, isOutput=False)
    outwt_d = nc.declare_dram_parameter("outwt", [C, C], f32, isOutput=False)
    outb_d = nc.declare_dram_parameter("outb", [1, C], f32, isOutput=False)
    out_d = nc.declare_dram_parameter("out", [B * QC, C], f32, isOutput=True)

    # A2A payload rows 0:128 = unnormalized O (2 heads); 128:130 = denoms
    binc = [nc.dram_tensor(f"binc{b}", [n_cores, 130, QC], bf16)
            for b in range(B)]
    bout = [nc.dram_tensor(f"bout{b}", [n_cores, 130, QC], bf16)
            for b in range(B)]

    with tile.TileContext(nc) as tc:
        with (
            tc.tile_pool(name="persist", bufs=1) as pp,
            tc.tile_pool(name="xload", bufs=2) as xp,
            tc.tile_pool(name="xt", bufs=2) as xtp,
            tc.tile_pool(name="pexp", bufs=3) as pexpp,
            tc.tile_pool(name="misc", bufs=2) as mp,
            tc.tile_pool(name="recvp", bufs=1) as rp,
            tc.tile_pool(name="scratch", bufs=2, space="PSUM") as scr,
            tc.tile_pool(name="stp", bufs=2, space="PSUM") as stp,
            tc.tile_pool(name="op", bufs=2, space="PSUM") as op,
        ):
            # ---- weights: load host-pre-transposed, cast to bf16 ----
            wT = [pp.tile([128, CS, 128], bf16, tag=f"wT{m}", name=f"wT{m}")
                  for m in range(3)]
            for m in range(3):
                wld = xp.tile([128, CS, 128], f32, tag="wld")
                nc.sync.dma_start(
                    wld[:],
                    qkvwt_d[m].rearrange("(cs p) d -> p cs d", p=128))
                nc.vector.tensor_copy(wT[m][:], wld[:])

            owT = pp.tile([128, CS, C], bf16, tag="owT")
            for g in range(CS):
                owld = xp.tile([128, C], f32, tag="owld")
                nc.sync.dma_start(owld[:], outwt_d[g * 128:(g + 1) * 128, :])
                nc.vector.tensor_copy(owT[:, g, :], owld[:])

            bias_sb = pp.tile([128, 3], f32, tag="bias")
            for m in range(3):
                nc.sync.dma_start(bias_sb[:, m:m + 1], qkvb_d[m])
            outb_f = pp.tile([1, C], f32, tag="outbf")
            nc.sync.dma_start(outb_f[:], outb_d[:])
            outb_sb = pp.tile([1, C], bf16, tag="outb")
            nc.vector.tensor_copy(outb_sb[:], outb_f[:])
            ones_sb = pp.tile([1, 128], bf16, tag="ones")
            nc.vector.memset(ones_sb[:], 1.0)

            # ---- per-batch persistent tensors ----
            qT = [pp.tile([128, NB], bf16, tag=f"qT{b}", name=f"qT{b}")
                  for b in range(B)]
            kT = [pp.tile([128, NB], bf16, tag=f"kT{b}", name=f"kT{b}")
                  for b in range(B)]
            # V layout per key-block: [h0 d0..63, ones, h1 d0..63, ones]
            # -> O-matmul output row 64 is the softmax denominator
            V = [pp.tile([128, NMB, 130], bf16, tag=f"V{b}", name=f"V{b}")
                 for b in range(B)]
            for b in range(B):
                nc.vector.memset(V[b][:], 1.0)

            qkv_state = {}

            def qkv_load(b, grp):
                tok0 = b * NB + grp * GRP
                xlf = xp.tile([128, CS, GRP], f32, tag="xlf")
                nc.sync.dma_start(
                    xlf[:],
                    xt_d[:, tok0:tok0 + GRP].rearrange(
                        "(cs p) t -> p cs t", p=128))
                xt = xtp.tile([128, CS, GRP], bf16, tag="xt")
                nc.vector.tensor_copy(xt[:], xlf[:])
                qkv_state[(b, grp)] = xt

            def qkv_proj(b, grp, m):
                xt = qkv_state[(b, grp)] if m < 2 else qkv_state.pop((b, grp))
                if True:
                    qp = scr.tile([128, GRP], f32, tag="s")
                    for cs in range(CS):
                        nc.tensor.matmul(
                            qp[:], wT[m][:, cs, :], xt[:, cs, :],
                            start=(cs == 0), stop=(cs == CS - 1))
                    if m < 2:
                        dest = (qT if m == 0 else kT)[b][
                            :, grp * GRP:(grp + 1) * GRP]
                        nc.vector.tensor_scalar(dest, qp[:],
                                                bias_sb[:, m:m + 1],
                                                None, op0=ALU.add)
                    else:
                        vs = mp.tile([128, GRP], bf16, tag="vs")
                        nc.vector.tensor_scalar(vs[:], qp[:],
                                                bias_sb[:, 2:3],
                                                None, op0=ALU.add)
                        mb0 = grp * (GRP // 128)
                        # xbar transpose (full-128-partition source), then
                        # two strided DVE copies split the heads into V
                        vt = mp.tile([128, GRP // 128, 128], bf16, tag="vt")
                        nc.sync.dma_start_transpose(vt[:], vs[:])
                        nc.vector.tensor_copy(
                            V[b][:, mb0:mb0 + 4, 0:64], vt[:, :, 0:64])
                        nc.vector.tensor_copy(
                            V[b][:, mb0:mb0 + 4, 65:129], vt[:, :, 64:128])

            def attn_phase(b, hooks=None):
                for qc in range(NQC):
                    oh0 = op.tile([65, QC], f32, tag="oh")
                    oh1 = op.tile([65, QC], f32, tag="oh")
                    # O-matmuls lag the S-matmuls by 2 iterations so the PE
                    # FIFO never stalls waiting for the exp of the current
                    # score tile (ACT ~1.1us > S-pair ~0.6us)
                    pes = {}
                    for mb in range(NMB + 2):
                        # foreign work (other batch's qkv, other batch's
                        # tail) is injected in small lumps the 2-deep st
                        # buffer can absorb without stalling ScalarE
                        if hooks and (qc, mb) in hooks:
                            for f in hooks[(qc, mb)]:
                                f()
                        if mb < NMB:
                            st = stp.tile([128, 2 * QC], f32, tag="st")
                            for h in range(2):
                                nc.tensor.matmul(
                                    st[:, h * QC:(h + 1) * QC],
                                    kT[b][64 * h:64 * h + 64,
                                          mb * 128:mb * 128 + 128],
                                    qT[b][64 * h:64 * h + 64,
                                          qc * QC:(qc + 1) * QC],
                                    start=True, stop=True)
                            pe = pexpp.tile([128, 2 * QC], bf16, tag="pe")
                            nc.scalar.activation(pe[:], st[:], AF.Exp,
                                                 scale=0.125)
                            pes[mb] = pe
                        if mb >= 2:
                            mo = mb - 2
                            pe = pes.pop(mo)
                            nc.tensor.matmul(oh0[:], V[b][:, mo, 0:65],
                                             pe[:, 0:QC],
                                             start=(mo == 0),
                                             stop=(mo == NMB - 1))
                            nc.tensor.matmul(oh1[:], V[b][:, mo, 65:130],
                                             pe[:, QC:2 * QC],
                                             start=(mo == 0),
                                             stop=(mo == NMB - 1))
                    for h, oh in ((0, oh0), (1, oh1)):
                        # evacuation DMAs ride the idle gpsimd queue so the
                        # sync queue (x loads) never stalls on attention
                        ohs = mp.tile([64, QC], bf16, tag="ohs")
                        nc.vector.tensor_copy(ohs[:], oh[0:64, :])
                        nc.gpsimd.dma_start(
                            binc[b][qc, h * 64:(h + 1) * 64, :], ohs[:])
                        ds = mp.tile([1, QC], bf16, tag="ds")
                        nc.vector.tensor_copy(ds[:], oh[64:65, :])
                        nc.gpsimd.dma_start(
                            binc[b][qc, 128 + h:129 + h, :], ds[:])

            def a2a_phase(b):
                nc.gpsimd.collective_compute(
                    "AllToAll", ALU.bypass,
                    replica_groups=[list(range(n_cores))],
                    ins=[binc[b].ap().opt()],
                    outs=[bout[b].ap().opt()],
                )

            tail_state = {}

            def tail_recv(b):
                recv = rp.tile([128, n_cores * QC], bf16, tag="recv")
                for g in range(n_cores):
                    nc.sync.dma_start(recv[:, g * QC:(g + 1) * QC],
                                      bout[b][g, 0:128, :])
                recvd = rp.tile([1, n_cores * 2 * QC], bf16, tag="recvd")
                nc.sync.dma_start(
                    recvd[:].rearrange("p (g h q) -> p g h q", g=n_cores, h=2),
                    bout[b][:, 128:130, :])
                rnorm = rp.tile([128, n_cores * QC], bf16, tag="rnorm")
                tail_state[b] = (recv, recvd, rnorm)

            def tail_norm_g(b, g):
                recv, recvd, rnorm = tail_state[b]
                # K=1 ones-matmul broadcasts the two denominator rows
                # across partitions 0:64 / 64:128 of a PSUM tile
                rb = scr.tile([128, QC], f32, tag="s")
                for h in range(2):
                    nc.tensor.matmul(
                        rb[h * 64:(h + 1) * 64, :],
                        ones_sb[0:1, 0:64],
                        recvd[0:1, (g * 2 + h) * QC:(g * 2 + h + 1) * QC],
                        start=True, stop=True)
                rcp = mp.tile([128, QC], f32, tag="rcp")
                nc.vector.reciprocal(rcp[:], rb[:])
                nc.vector.scalar_tensor_tensor(
                    rnorm[:, g * QC:(g + 1) * QC],
                    recv[:, g * QC:(g + 1) * QC], 1.0, rcp[:],
                    op0=ALU.mult, op1=ALU.mult)

            ot_state = {}

            def tail_out_g(b, tb, co2):
                rnorm = tail_state[b][2]
                if co2 == 0:
                    ot = mp.tile([128, C], f32, tag="ot", name=f"ot{b}_{tb}")
                    ot_state[(b, tb)] = ot
                ot = ot_state[(b, tb)]
                pj = scr.tile([128, 512], f32, tag="s")
                for g in range(n_cores):
                    nc.tensor.matmul(
                        pj[:],
                        rnorm[:, g * QC + tb * 128:g * QC + tb * 128 + 128],
                        owT[:, g, co2 * 512:(co2 + 1) * 512],
                        start=(g == 0), stop=False)
                nc.tensor.matmul(pj[:], ones_sb[:],
                                 outb_sb[:, co2 * 512:(co2 + 1) * 512],
                                 start=False, stop=True)
                nc.vector.tensor_copy(ot[:, co2 * 512:(co2 + 1) * 512], pj[:])
                if co2 == C // 512 - 1:
                    del ot_state[(b, tb)]
                    nc.sync.dma_start(
                        out_d[b * QC + tb * 128:b * QC + (tb + 1) * 128, :],
                        ot[:])

            from functools import partial

            hooks0 = {}
            for g in range(NGRP):
                hooks0[(g, 0)] = [partial(qkv_load, 1, g),
                                  partial(qkv_proj, 1, g, 0)]
                hooks0[(g, 11)] = [partial(qkv_proj, 1, g, 1)]
                hooks0[(g, 22)] = [partial(qkv_proj, 1, g, 2)]

            hooks1 = {(2, 16): [partial(tail_recv, 0)]}
            for g in range(n_cores):
                hooks1[(3, 4 * g)] = [partial(tail_norm_g, 0, g)]
            for i in range(OTB * (C // 512)):
                tb, co2 = divmod(i, C // 512)
                qc, mb = 4 + i // 4, (i % 4) * 8
                hooks1[(qc, mb)] = [partial(tail_out_g, 0, tb, co2)]

            for grp in range(NGRP):
                qkv_load(0, grp)
                for m in range(3):
                    qkv_proj(0, grp, m)
            attn_phase(0, hooks=hooks0)
            a2a_phase(0)
            attn_phase(1, hooks=hooks1)
            a2a_phase(1)
            tail_recv(1)
            for g in range(n_cores):
                tail_norm_g(1, g)
            for tb in range(OTB):
                for co2 in range(C // 512):
                    tail_out_g(1, tb, co2)

    nc.compile()
    return nc


def shard_inputs(x, qkv_w, qkv_b, out_w, out_b, n_cores=8):
    """Per-core input maps. hidden == 128*n_cores; core c owns qkv rows
    [c*128, (c+1)*128) of each of q, k, v. x and the weights are
    pre-transposed on the host so the device needs no transposes."""
    Bv, N, Cc = x.shape
    T = Bv * N
    xth = np.ascontiguousarray(x.reshape(T, Cc).T, dtype=np.float32)
    owt = np.ascontiguousarray(out_w.T, dtype=np.float32)
    ob = np.ascontiguousarray(out_b.reshape(1, Cc), dtype=np.float32)
    in_maps = []
    for c in range(n_cores):
        r0 = c * 128
        wt = np.stack([
            np.ascontiguousarray(qkv_w[m * Cc + r0: m * Cc + r0 + 128].T)
            for m in range(3)])
        bvec = np.stack([qkv_b[m * Cc + r0: m * Cc + r0 + 128]
                         for m in range(3)])[:, :, None]
        in_maps.append({
            "xt": xth,
            "qkvwt": np.ascontiguousarray(wt.astype(np.float32)),
            "qkvb": np.ascontiguousarray(bvec.astype(np.float32)),
            "outwt": owt, "outb": ob,
        })
    return in_maps


def unshard_output(results, n_cores=8, n_tok_b=4096, hidden=1024):
    """results[c]["out"] is [B*512, C]: rows 0:512 = batch-0 slice c,
    rows 512:1024 = batch-1 slice c."""
    QC = 512
    out = np.empty((B, n_tok_b, hidden), dtype=np.float32)
    for c in range(n_cores):
        r = results[c]["out"]
        for b in range(B):
            out[b, c * QC:(c + 1) * QC] = r[b * QC:(b + 1) * QC]
    return out


_NC_CACHE = {}


def kernel(x, qkv_w, qkv_b, out_w, out_b):
    from concourse import bass_utils
    x = np.asarray(x)
    Bv, N, Cc = x.shape
    key = (N, Cc)
    if key not in _NC_CACHE:
        _NC_CACHE[key] = build_nc(n_tok_b=N, n_cores=N_CORES, hidden=Cc)
    nc = _NC_CACHE[key]
    in_maps = shard_inputs(x, np.asarray(qkv_w), np.asarray(qkv_b),
                           np.asarray(out_w), np.asarray(out_b),
                           n_cores=N_CORES)
    res = bass_utils.run_bass_kernel_spmd(nc, in_maps,
                                          core_ids=list(range(N_CORES)))
    out = unshard_output([res.results[i] for i in range(N_CORES)],
                         n_cores=N_CORES, n_tok_b=N, hidden=Cc)
    return out.astype(np.float32)
